# revision 24
# baseline (speedup 1.0000x reference)
"""Tensor-parallel attention kernel for 8 Trainium2 NeuronCores.

Reference computation (S=2048, B=2, H=2048, NH=16 heads, HD=128):
    q = x @ wq.T ; k = x @ wk.T ; v = x @ wv.T          (x: [S, B, H])
    per (b, head): out = softmax(q k^T / sqrt(HD)) v
    return concat_heads(out) @ wo.T                      ([S, B, H])

Sharding: tensor-parallel over heads; core r owns heads {2r, 2r+1}. Combine
before the output projection via AllToAll of bf16 attention outputs; each core
applies the full wo to its 512-token slice.

Schedule (v2): the PE stream is the bottleneck (786k cycles ~= 328us at
2.4GHz), and the PE only reaches 2.4GHz after 3us of continuous execution —
so the emission order interleaves every ACT-bound attention stretch with
projection / output-projection matmuls:

  E1   ph1 sweeps nb0-2 (k0,q0,v,k1,q1 per nb; x tile freed per sweep)
  E2   nb3 sweep + b0h0 attention units interleaved, then nb4 (b=1) sweep
       as filler inside the ACT-bound stretch
  E3   nb5-7 sweeps + b0h1 units interleaved (PE-rich era)
  E4   b1h0 units (paced by exp; leftover chunks as filler), A2A#0,
       or0 load, then b1h1 units with ph3a groups as filler once or0 lands
  E5   ph3a remainder during A2A#1
  E6   ph3b gated on or1 chunks + out writes

Attention units are generators yielding between g-groups; pv matmuls lag the
scores by 2 groups so exp (1.15us/group on ACT) never stalls the PE.
"""

import numpy as np

S, B, H = 2048, 2, 2048
NH, HD = 16, 128
N_CORES = 8
HPC = NH // N_CORES          # heads per core (2)
FPC = HPC * HD               # features per core (256)
NT = S * B                   # tokens (4096)
SCALE = HD ** -0.5
KT = H // 128                # contraction tiles (16)
NB = 512                     # token block width in phase 1
XW = KT * NB                 # x big-tile width (8192)
QT = 512                     # q-tile width in phase 2
EXPW = 1024                  # exp batch width (2 key-blocks per ACT op)
JB = S // 128                # key blocks per (b, h) (16)
LAG = 2                      # pv lags scores by this many g-groups


def _build():
    import concourse.mybir as mybir
    import concourse.tile as tile
    from concourse import bacc

    F32 = mybir.dt.float32
    BF16 = mybir.dt.bfloat16
    Exp = mybir.ActivationFunctionType.Exp

    nc = bacc.Bacc(None, target_bir_lowering=False, num_devices=N_CORES)

    # Pre-shuffled inputs (see make_in_maps):
    #   xS[nb*128+p, kt*NB+t] = x_bf16[feature kt*128+p, token nb*NB+t]
    #   wq/wkS[p, m*2048 + kt*128 + f] = w.T[kt*128+p, m*128+f]  (m-major)
    #   wvS[p, kt*FPC+f]      = wv.T[kt*128+p, f]
    #   woS[(nt*2+par)*128+p, j*512+t] = wo.T[(2j+par)*128+p, nt*512+t]
    xS = nc.dram_tensor("xS", [8 * 128, XW], BF16, kind="ExternalInput")
    wqS = nc.dram_tensor("wqS", [128, KT * FPC], BF16, kind="ExternalInput")
    wkS = nc.dram_tensor("wkS", [128, KT * FPC], BF16, kind="ExternalInput")
    wvS = nc.dram_tensor("wvS", [128, KT * FPC], BF16, kind="ExternalInput")
    woS = nc.dram_tensor("woS", [8 * 128, 8 * 512], BF16, kind="ExternalInput")
    out = nc.dram_tensor("out", [NT // N_CORES, H], F32, kind="ExternalOutput")

    from contextlib import ExitStack

    with tile.TileContext(nc) as tc, ExitStack() as ctx:
        pool = lambda **kw: ctx.enter_context(tc.tile_pool(**kw))
        qk_res = pool(name="qk_res", bufs=1)
        v_res = pool(name="v_res", bufs=32)
        const = pool(name="const", bufs=1)
        x_p = pool(name="x_p", bufs=3)
        w_p1 = pool(name="w_p1", bufs=1)
        wo_p = pool(name="wo_p", bufs=5)
        p_p2 = pool(name="p_p2", bufs=4)
        acc_p = pool(name="acc_p", bufs=2)
        r_p2 = pool(name="r_p2", bufs=1)
        ost_p = pool(name="ost_p", bufs=2)
        or_p = pool(name="or_p", bufs=1)
        part_p = pool(name="part_p", bufs=16)
        ev_p3 = pool(name="ev_p3", bufs=2)
        ps_qk = pool(name="ps_qk", bufs=2, space="PSUM")
        ps_sc = pool(name="ps_sc", bufs=2, space="PSUM")
        ps_pv = pool(name="ps_pv", bufs=2, space="PSUM")
        dram = pool(name="dram", bufs=1, space="DRAM")

        ones_f = const.tile([128, 128], F32)
        nc.vector.memset(ones_f[:], 1.0)
        ones = const.tile([128, 128], BF16)
        nc.vector.tensor_copy(ones[:], ones_f[:])

        qhat = [qk_res.tile([128, NT], BF16, tag=f"q{m}", name=f"qhat{m}")
                for m in range(2)]
        khat = [qk_res.tile([128, NT], BF16, tag=f"k{m}", name=f"khat{m}")
                for m in range(2)]
        vsb = [v_res.tile([128, FPC], BF16, tag="v", name=f"vsb{i}")
               for i in range(NT // 128)]
        o_send = [dram.tile([8 * 128, QT], BF16, name=f"o_send{h}")
                  for h in range(2)]
        o_recv = [dram.tile([8 * 128, QT], BF16, name=f"o_recv{h}")
                  for h in range(2)]

        # ---- input loads -------------------------------------------------
        # wk-m0 first on scalar (the first PE work is the k0 group of nb0);
        # x0 quarters split sync+scalar (the two fast HWDGE queues); wv on
        # gpsimd which is otherwise idle at start; x stripes sync/gpsimd.
        wk_all = w_p1.tile([128, KT * FPC], BF16, tag="wk", name="wk_all")
        wq_all = w_p1.tile([128, KT * FPC], BF16, tag="wq", name="wq_all")
        wv_all = w_p1.tile([128, KT * FPC], BF16, tag="wv", name="wv_all")
        HW = KT * FPC // 2  # one m-half (2048 cols)
        nc.scalar.dma_start(wk_all[:, 0:HW // 2], wkS[:, 0:HW // 2])
        x0 = x_p.tile([128, XW], BF16, tag="x", name="x0")
        nc.sync.dma_start(x0[:, 0:XW // 4], xS[0:128, 0:XW // 4])
        nc.scalar.dma_start(x0[:, XW // 2:3 * XW // 4],
                            xS[0:128, XW // 2:3 * XW // 4])
        nc.sync.dma_start(x0[:, XW // 4:XW // 2], xS[0:128, XW // 4:XW // 2])
        nc.scalar.dma_start(wk_all[:, HW // 2:HW], wkS[:, HW // 2:HW])
        nc.sync.dma_start(x0[:, 3 * XW // 4:], xS[0:128, 3 * XW // 4:XW])
        nc.scalar.dma_start(wq_all[:, 0:HW], wqS[:, 0:HW])
        nc.scalar.dma_start(wv_all[:, 0:KT * FPC // 2],
                            wvS[:, 0:KT * FPC // 2])
        nc.scalar.dma_start(wv_all[:, KT * FPC // 2:], wvS[:, KT * FPC // 2:])
        nc.scalar.dma_start(wk_all[:, HW:], wkS[:, HW:])
        nc.scalar.dma_start(wq_all[:, HW:], wqS[:, HW:])

        x_big = {0: x0}

        def load_x(nb, eng):
            t = x_p.tile([128, XW], BF16, tag="x", name=f"x{nb}")
            eng.dma_start(t[:], xS[nb * 128:(nb + 1) * 128, :])
            x_big[nb] = t

        def xt(nb, kt):
            return x_big[nb][:, kt * NB:(kt + 1) * NB]

        # m-major weight tile accessors
        def wqk_t(w_all, m, kt):
            base = m * (KT * 128) + kt * 128
            return w_all[:, base:base + 128]

        wv_t = [wv_all[:, kt * FPC:(kt + 1) * FPC] for kt in range(KT)]

        # ---- phase-1 chunk emitters (~0.85us each on the PE) -------------
        def qk_halves(kind, m, nb):
            """One q/k projection group split into two chunk closures that
            share a psum tile (unit matmuls interleave between halves on
            other psum tags, so the accumulation group stays open safely)."""
            w_all = wk_all if kind == "k" else wq_all
            dest = khat[m] if kind == "k" else qhat[m]
            cell = {}

            def first():
                cell["ps"] = ps_qk.tile([128, NB], F32, tag="qk",
                                        name="qk_ps")
                for kt in range(KT // 2):
                    nc.tensor.matmul(
                        cell["ps"][:], wqk_t(w_all, m, kt), xt(nb, kt),
                        start=(kt == 0), stop=False,
                    )

            def second():
                ps = cell["ps"]
                for kt in range(KT // 2, KT):
                    nc.tensor.matmul(
                        ps[:], wqk_t(w_all, m, kt), xt(nb, kt),
                        start=False, stop=(kt == KT - 1),
                    )
                nc.vector.tensor_copy(dest[:, nb * NB:(nb + 1) * NB], ps[:])

            return [first, second]

        def v_group(nb, sub):
            ps = ps_qk.tile([128, FPC], F32, tag="qk")
            for kt in range(KT):
                nc.tensor.matmul(
                    ps[:], xt(nb, kt)[:, sub * 128:(sub + 1) * 128], wv_t[kt],
                    start=(kt == 0), stop=(kt == KT - 1),
                )
            nc.vector.tensor_copy(vsb[nb * 4 + sub][:], ps[:])

        def v_chunks(nb):
            return [lambda nb=nb, s=sub: v_group(nb, s) for sub in range(4)]

        def kq_chunks(nb, ms):
            ch = []
            for kind, m in ms:
                ch += qk_halves(kind, m, nb)
            return ch

        def emit(chunks):
            for chf in chunks:
                chf()

        # ---- wo loads (scalar queue, after qkv weights) ------------------
        wo_t = [[[None, None], [None, None]] for _ in range(4)]

        def load_wo(nt, par, half, eng):
            t = wo_p.tile([128, 4 * 512], BF16, tag="wo",
                          name=f"wo{nt}_{par}_{half}")
            r0 = (nt * 2 + par) * 128
            eng.dma_start(
                t[:], woS[r0:r0 + 128, half * 2048:(half + 1) * 2048]
            )
            wo_t[nt][par][half] = t

        # ---- attention unit generator -----------------------------------
        # The unit's tail (last two pv groups + denominator sum + recip +
        # ostg + o_send) is deferred into the NEXT unit's second iteration:
        # by then exp(g7) and the DVE adds are long done, so the PE never
        # waits (a wait would reset the 3us pstate ramp and halve the PE
        # clock for the next stretch).
        def unit_gen(b, h, qt, tails):
            q_bh = qhat[h][:, b * S + qt * QT: b * S + (qt + 1) * QT]
            pv_ps = ps_pv.tile([128, QT], F32, tag="pv")
            acc = acc_p.tile([128, QT], BF16, tag="acc")
            pTs = {}

            def do_pv(g):
                pT = pTs.pop(g)
                for i in range(2):
                    jb = g * 2 + i
                    nc.tensor.matmul(
                        pv_ps[:],
                        vsb[b * JB + jb][:, h * 128:(h + 1) * 128],
                        pT[:, i * QT:(i + 1) * QT],
                        start=(jb == 0), stop=(jb == JB - 1),
                    )

            for g in range(JB // 2):
                sc_ps = ps_sc.tile([128, EXPW], F32, tag="sc")
                pT = p_p2.tile([128, EXPW], BF16, tag="p")
                for i in range(2):
                    jb = g * 2 + i
                    nc.tensor.matmul(
                        sc_ps[:, i * QT:(i + 1) * QT],
                        khat[h][:, b * S + jb * 128: b * S + (jb + 1) * 128],
                        q_bh, start=True, stop=True,
                    )
                nc.scalar.activation(pT[:], sc_ps[:], Exp, scale=SCALE)
                if g == 0:
                    nc.vector.tensor_add(acc[:], pT[:, 0:QT], pT[:, QT:EXPW])
                else:
                    nc.vector.tensor_add(acc[:], acc[:], pT[:, 0:QT])
                    nc.vector.tensor_add(acc[:], acc[:], pT[:, QT:EXPW])
                pTs[g] = pT
                if g == 1 and tails:
                    tails.pop()()
                if g >= LAG:
                    do_pv(g - LAG)
                yield

            def tail():
                for g in range(JB // 2 - LAG, JB // 2):
                    do_pv(g)
                sum_ps = ps_qk.tile([128, QT], F32, tag="qk")
                nc.tensor.matmul(sum_ps[:], ones[:], acc[:],
                                 start=True, stop=True)
                recip = r_p2.tile([128, QT], F32, tag="r")
                nc.vector.reciprocal_approx_fast(recip[:], sum_ps[:])
                ostg = ost_p.tile([128, QT], BF16, tag="ost")
                nc.vector.tensor_mul(ostg[:], pv_ps[:], recip[:])
                c = b * (S // QT) + qt
                # scalar queue: weight/wo transfers drain well before the
                # last o_send write of each head, so the A2A fires asap.
                nc.scalar.dma_start(
                    o_send[h][c * 128:(c + 1) * 128, :], ostg[:])

            tails.append(tail)
            yield

        def drive(uargs, fillers, front=0):
            """Emit an era: unit generators sequentially, one filler chunk
            per yield for the first `front` yields, the rest spread evenly.
            Deferred tails chain between units; the era's last tail is
            emitted at the end (before any A2A trigger that needs it)."""
            tails = []
            gens = [unit_gen(b, h, qt, tails) for (b, h, qt) in uargs]
            total = 9 * len(gens)
            done = 0
            fi = 0
            for ug in gens:
                for _ in ug:
                    done += 1
                    if done <= front:
                        if fi < len(fillers):
                            fillers[fi]()
                            fi += 1
                    else:
                        rest = len(fillers) - min(front, len(fillers))
                        target = min(front, len(fillers)) + (
                            rest * (done - front)) // max(total - front, 1)
                        while fi < target:
                            fillers[fi]()
                            fi += 1
            while fi < len(fillers):
                fillers[fi]()
                fi += 1
            while tails:
                tails.pop()()

        # ---- collectives -------------------------------------------------
        def a2a(h):
            nc.gpsimd.collective_compute(
                "AllToAll",
                mybir.AluOpType.bypass,
                replica_groups=[list(range(N_CORES))],
                ins=[o_send[h][:].opt()],
                outs=[o_recv[h][:].opt()],
            )

        def load_or(h):
            engines = [nc.gpsimd] if h == 0 else [nc.sync, nc.scalar]
            nch = 4 if h == 0 else 8
            w = 4096 // nch
            t = or_p.tile([128, 8 * 512], BF16, tag=f"or{h}", name=f"or{h}")
            for chi in range(nch):
                engines[chi % len(engines)].dma_start(
                    t[:, chi * w:(chi + 1) * w].rearrange(
                        "p (j t) -> p j t", j=w // 512),
                    o_recv[h][chi * (w // 4):(chi + 1) * (w // 4), :].rearrange(
                        "(j p) t -> p j t", p=128),
                )
            return t

        or_big = [None, None]
        parts = {}

        # ---- phase-3 group emitters -------------------------------------
        def ph3a_group(nt, tb):
            ps = ps_qk.tile([128, 512], F32, tag="qk")
            for j in range(8):
                nc.tensor.matmul(
                    ps[:],
                    or_big[0][:, j * 512 + tb * 128: j * 512 + (tb + 1) * 128],
                    wo_t[nt][0][j // 4][:, (j % 4) * 512:(j % 4 + 1) * 512],
                    start=(j == 0), stop=(j == 7),
                )
            part = part_p.tile([128, 512], BF16, tag="part")
            nc.vector.tensor_copy(part[:], ps[:])
            parts[nt, tb] = part

        def ph3b_group(nt, tb):
            ps = ps_qk.tile([128, 512], F32, tag="qk")
            for j in range(8):
                nc.tensor.matmul(
                    ps[:],
                    or_big[1][:, j * 512 + tb * 128: j * 512 + (tb + 1) * 128],
                    wo_t[nt][1][j // 4][:, (j % 4) * 512:(j % 4 + 1) * 512],
                    start=(j == 0), stop=(j == 7),
                )
            ev = ev_p3.tile([128, 512], F32, tag="ev")
            nc.vector.tensor_add(ev[:], ps[:], parts[nt, tb][:])
            nc.sync.dma_start(
                out[tb * 128:(tb + 1) * 128, nt * 512:(nt + 1) * 512],
                ev[:],
            )

        # ================= main schedule =================================
        uq = lambda b, h: [(b, h, qt) for qt in range(S // QT)]
        ALL_KQ = [("k", 0), ("q", 0), ("k", 1), ("q", 1)]

        # warmup collective: absorbs the ~11.5us first-collective trigger
        # delay long before A2A#0 needs the CC stream.
        warm = dram.tile([8 * 128, 8], BF16, name="warm_buf")
        warm_o = dram.tile([8 * 128, 8], BF16, name="warm_out")
        nc.gpsimd.collective_compute(
            "AllToAll", mybir.AluOpType.bypass,
            replica_groups=[list(range(N_CORES))],
            ins=[warm[:].opt()], outs=[warm_o[:].opt()],
        )

        # E1: nb0-2 full sweeps (k0,q0 first, then v, k1, q1)
        load_x(1, nc.sync)
        load_x(2, nc.gpsimd)
        for nb in (0, 1, 2):
            emit(kq_chunks(nb, ALL_KQ[:2]))
            emit(v_chunks(nb))
            emit(kq_chunks(nb, ALL_KQ[2:]))
            load_x(nb + 3, (nc.sync, nc.gpsimd)[nb % 2])
        # E2pre: nb3 k0/q0/v (b0h0 needs khat0/qhat0/vsb of all b=0)
        emit(kq_chunks(3, ALL_KQ[:2]))
        emit(v_chunks(3))
        # wo par0 loads queue on scalar here (transfers overlap attention)
        for nt in range(4):
            for half in range(2):
                load_wo(nt, 0, half, nc.scalar)

        # E2: b0h0 units; filler = k1/q1(nb3) + v(nb4)
        drive(uq(0, 0), kq_chunks(3, ALL_KQ[2:]) + v_chunks(4))
        load_x(6, nc.gpsimd)

        # E3: pure phase-1 stretch: rest of nb4, k0/q0 of nb5-7, v(nb6)
        # (ACT idles here; the PE runs flat out)
        emit(kq_chunks(4, ALL_KQ))
        load_x(7, nc.sync)
        emit(kq_chunks(5, ALL_KQ[:2]))
        emit(kq_chunks(6, ALL_KQ[:2]))
        emit(v_chunks(6))
        emit(kq_chunks(7, ALL_KQ[:2]))

        # E4: b1h0 units (before b0h1, so A2A#0 fires early and its or0
        # data lands well before phase 3a needs it). Filler: v(nb5)+v(nb7)
        # front-loaded at 1/yield (the unit pv's consume vsb(b1) blocks in
        # jb order: nb5 feeds g2-3, nb7 feeds g6-7), then k1/q1(nb7).
        drive(uq(1, 0),
              v_chunks(5) + v_chunks(7) + kq_chunks(7, ALL_KQ[2:]),
              front=8)
        a2a(0)
        or_big[0] = load_or(0)
        # wo par1-h0 loads (needed for ph3b only); par1-h1 must wait until
        # the wo ring's par0 slots free up (ph3a reads them), so those load
        # on sync after or1 — loading them here deadlocks the scalar queue
        # behind slot-waits that need exps queued after them.
        for nt in range(4):
            load_wo(nt, 1, 0, nc.scalar)

        # E5: b0h1 units; filler = k1/q1 of nb5+nb6 (needed by b1h1 only)
        drive(uq(0, 1), kq_chunks(5, ALL_KQ[2:]) + kq_chunks(6, ALL_KQ[2:]))

        # E6: b1h1 units, no filler (nothing independent remains; ph3a is
        # kept off the PE queue until after the A2A#1 trigger so an or0
        # dependency can never head-of-line-block the units)
        drive(uq(1, 1), [])
        a2a(1)
        or_big[1] = load_or(1)
        for nt in range(4):
            load_wo(nt, 1, 1, nc.sync)

        # E7: all of ph3a during A2A#1 (or0 landed long ago)
        for nt in range(4):
            for tb in range(4):
                ph3a_group(nt, tb)

        # E8: ph3b + out writes (or1 chunks arrive in j order)
        for nt in range(4):
            for tb in range(4):
                ph3b_group(nt, tb)
    nc.compile()
    return nc


_NC_CACHE = None


def _get_nc():
    global _NC_CACHE
    if _NC_CACHE is None:
        _NC_CACHE = _build()
    return _NC_CACHE


def make_in_maps(x, wq, wk, wv, wo):
    import ml_dtypes

    bf = ml_dtypes.bfloat16
    x = np.asarray(x, dtype=np.float32)
    # tokens b-major: t = b*S + s
    xT = np.ascontiguousarray(x.transpose(2, 1, 0).reshape(H, NT))
    xS = np.ascontiguousarray(
        xT.reshape(KT, 128, 8, NB).transpose(2, 1, 0, 3).reshape(8 * 128, XW)
    ).astype(bf)
    woT = np.asarray(wo, dtype=np.float32).T  # [f_in, f_out]
    woS = np.ascontiguousarray(
        woT.reshape(8, 2, 128, 4, 512).transpose(3, 1, 2, 0, 4).reshape(
            8 * 128, 8 * 512)
    ).astype(bf)

    def wshuf_mmaj(w, r):
        # m-major: wS[p, m*2048 + kt*128 + f] = w.T[kt*128+p, m*128+f]
        sl = slice(r * FPC, (r + 1) * FPC)
        wT = np.asarray(w, dtype=np.float32)[sl, :].T  # [H, FPC]
        return np.ascontiguousarray(
            wT.reshape(KT, 128, 2, 128).transpose(1, 2, 0, 3).reshape(
                128, KT * FPC)
        ).astype(bf)

    def wshuf_kt(w, r):
        sl = slice(r * FPC, (r + 1) * FPC)
        wT = np.asarray(w, dtype=np.float32)[sl, :].T  # [H, FPC]
        return np.ascontiguousarray(
            wT.reshape(KT, 128, FPC).transpose(1, 0, 2).reshape(128, KT * FPC)
        ).astype(bf)

    in_maps = []
    for r in range(N_CORES):
        in_maps.append(
            {
                "xS": xS,
                "wqS": wshuf_mmaj(wq, r),
                "wkS": wshuf_mmaj(wk, r),
                "wvS": wshuf_kt(wv, r),
                "woS": woS,
            }
        )
    return in_maps


def assemble_out(results):
    out_bs = np.concatenate([results[r]["out"] for r in range(N_CORES)], axis=0)
    return np.ascontiguousarray(out_bs.reshape(B, S, H).transpose(1, 0, 2))


def kernel(x, wq, wk, wv, wo):
    from concourse.bass_utils import run_bass_kernel_spmd

    in_maps = make_in_maps(x, wq, wk, wv, wo)
    res = run_bass_kernel_spmd(_get_nc(), in_maps, list(range(N_CORES)))
    return assemble_out(res.results)
